# revision 44
# baseline (speedup 1.0000x reference)
"""Trainium2 Bass kernel for nn_DmTranslateTrain (seq2seq translate train step).

Strategy (8 NeuronCores, SPMD):
  - Data-parallel over batch: core k owns batches [4k, 4k+4). Each core runs the
    full encoder LSTM scan + decoder (LSTM + Luong attention) for its 4 batches.
  - Output projection is tensor-parallel over the vocabulary: chunked AllGather
    of attention activations overlapped with the decoder, then each core
    computes logits[:, 4000k:4000k+4000].

Scan-step design (the hot loop):
  - The x-projection (emb @ Wx + b, precomputed in DRAM) is folded into the PE
    accumulation with a tiny K=4 identity matmul, so the gate nonlinearities
    read PSUM directly (no vector adds on the critical path).
  - All four gates use plain tanh: sigma(x) = (1+tanh(x/2))/2, with the 0.5
    pre-scale for gates i/f/o folded into the weight columns host-side.  One
    fused tanh per 512-col gate pair (2 ACTs per step), one activation table.
  - State kept scaled: C2 = 2c, H = 2h.  Updates via scalar_tensor_tensor:
      IG2 = (ti+1)*tu; FC2 = (tf+1)*C2; C2' = 0.5*FC2 + IG2;
      tc = tanh(0.5*C2'); H = (to+1)*tc.
  - z matmuls emitted kk-outer / m-inner so the 4 PE column-groups
    (tile_position=(0,32m)) stream concurrently; decoder scores run in column
    group q96 on separate PSUM partitions.

Gate packing: z tile is [128, 1024] per band m (partition = 32*m + b), free
col = gate*256 + 32*fc + r for unit u = 128*fc + 32*m + r, gates ordered
[u, i, f, o] (u = candidate).  The DVE 32x32 block transpose of the H tile
directly yields H^T in natural u-major chunks (one copy per step).
Logits rows are ordered (core, t, local batch); the host unshards.
"""

import numpy as np

B, TS, TD = 32, 64, 63
VS, VT = 32000, 32000
E, U = 256, 1024
G4 = 4 * U
NB = 4            # batches per core
NC = 8            # cores
VSH = VT // NC    # vocab shard per core
RE = TS * NB      # encoder rows per core
RD = TD * NB      # decoder rows per core
RT = TD * B       # total decoder rows (all batches)

_GATE_PERM = [2, 0, 1, 3]  # new order [u, i, f, o] -> original gate index
CHUNKS = [(0, 16), (16, 32), (32, 48), (48, 60), (60, TD)]


def _reorder_cols(w):
    # natural col = gate_orig*1024 + u, u = 128*fc + 32*m + r
    w5 = w.reshape(w.shape[0], 4, 8, 4, 32)        # [in, g_orig, fc, m, r]
    w5 = w5[:, _GATE_PERM]                          # [in, g_new, fc, m, r]
    w5 = w5.transpose(0, 3, 1, 2, 4)                # [in, m, g_new, fc, r]
    return np.ascontiguousarray(w5.reshape(w.shape[0], G4))


def _reorder_bias(b):
    b5 = b.reshape(4, 8, 4, 32)[_GATE_PERM].transpose(2, 0, 1, 3)
    return np.ascontiguousarray(b5.reshape(1, G4))


def _prep_host(inputs):
    import ml_dtypes
    bf16 = ml_dtypes.bfloat16
    f32 = np.float32
    enc_in = np.asarray(inputs["encoder_input"])
    dec_in = np.asarray(inputs["decoder_input"])
    Wx_e = np.asarray(inputs["Wx_e"], f32)
    Wh_e = np.asarray(inputs["Wh_e"], f32)
    b_e = np.asarray(inputs["b_e"], f32)
    Wx_d = np.asarray(inputs["Wx_d"], f32)
    Wh_d = np.asarray(inputs["Wh_d"], f32)
    b_d = np.asarray(inputs["b_d"], f32)
    Wm = np.asarray(inputs["Wm"], f32)
    Wa = np.asarray(inputs["Wa"], f32)
    Wf = np.asarray(inputs["Wf"], f32)
    bfv = np.asarray(inputs["bf"], f32)

    Wxd_x = Wx_d[:E]
    Wxd_a = Wx_d[E:]
    Wa_h, Wa_c = Wa[:U], Wa[U:]

    # per-gate column scale on the NATURAL layout (i, f, g, o): tanh trick
    # needs 0.5*z for i/f/o; the candidate gate g keeps full scale.
    cs = np.concatenate([np.full(U, 0.5, f32), np.full(U, 0.5, f32),
                         np.ones(U, f32), np.full(U, 0.5, f32)])

    shared = {
        "Wxe": _reorder_cols(Wx_e * cs).astype(bf16),
        "Whe": _reorder_cols(0.5 * Wh_e * cs).astype(bf16),
        "Whcomb": _reorder_cols(0.5 * (Wh_d + Wa_h @ Wxd_a) * cs).astype(bf16),
        "Wca": _reorder_cols(0.5 * (Wa_c @ Wxd_a) * cs).astype(bf16),
        "Whd0": _reorder_cols(0.5 * Wh_d * cs).astype(bf16),
        "Wxdx": _reorder_cols(Wxd_x * cs).astype(bf16),
        "Wm": (0.25 * Wm).astype(bf16),
        "WaH": (0.5 * Wa_h).astype(bf16),
        "WaC": np.ascontiguousarray((0.5 * Wa_c).astype(bf16)),
        "be": _reorder_bias(b_e * cs),
        "bd": _reorder_bias(b_d * cs),
        "enc_emb": np.ascontiguousarray(np.asarray(inputs["enc_emb"], f32)),
        "dec_emb": np.ascontiguousarray(np.asarray(inputs["dec_emb"], f32)),
    }
    Wf_bf = Wf.astype(bf16)
    per_core = []
    for k in range(NC):
        eidx = enc_in[NB * k:NB * (k + 1)]
        didx = dec_in[NB * k:NB * (k + 1)]
        per_core.append({
            "enc_idx": np.ascontiguousarray(eidx.T.reshape(RE, 1).astype(np.int32)),
            "dec_idx": np.ascontiguousarray(didx.T.reshape(RD, 1).astype(np.int32)),
            "Wfs": np.ascontiguousarray(Wf_bf[:, VSH * k:VSH * (k + 1)]),
            "bfs": np.ascontiguousarray(bfv[VSH * k:VSH * (k + 1)].reshape(1, VSH)),
        })
    return shared, per_core


# ---------------------------------------------------------------------------

def _build_nc(stage="full", debug=False):
    import re as _re
    from contextlib import ExitStack
    import concourse.bass as bass
    import concourse.mybir as mybir
    import concourse.tile as tile
    from concourse import bacc
    from concourse.masks import make_identity

    dt = mybir.dt
    AF = mybir.ActivationFunctionType
    ALU = mybir.AluOpType
    AX = mybir.AxisListType
    f32, bf = dt.float32, dt.bfloat16

    nc = bacc.Bacc("TRN2", target_bir_lowering=False, debug=False, num_devices=NC)

    enc_idx = nc.dram_tensor("enc_idx", [RE, 1], dt.int32, kind="ExternalInput")
    dec_idx = nc.dram_tensor("dec_idx", [RD, 1], dt.int32, kind="ExternalInput")
    enc_emb = nc.dram_tensor("enc_emb", [VS, E], f32, kind="ExternalInput")
    dec_emb = nc.dram_tensor("dec_emb", [VT, E], f32, kind="ExternalInput")
    Wxe = nc.dram_tensor("Wxe", [E, G4], bf, kind="ExternalInput")
    Whe = nc.dram_tensor("Whe", [U, G4], bf, kind="ExternalInput")
    Whcomb = nc.dram_tensor("Whcomb", [U, G4], bf, kind="ExternalInput")
    Wca_t = nc.dram_tensor("Wca", [U, G4], bf, kind="ExternalInput")
    Whd0 = nc.dram_tensor("Whd0", [U, G4], bf, kind="ExternalInput")
    Wxdx = nc.dram_tensor("Wxdx", [E, G4], bf, kind="ExternalInput")
    Wm_t = nc.dram_tensor("Wm", [U, U], bf, kind="ExternalInput")
    WaH_t = nc.dram_tensor("WaH", [U, U], bf, kind="ExternalInput")
    WaC_t = nc.dram_tensor("WaC", [U, U], bf, kind="ExternalInput")
    Wfs = nc.dram_tensor("Wfs", [U, VSH], bf, kind="ExternalInput")
    bfs = nc.dram_tensor("bfs", [1, VSH], f32, kind="ExternalInput")
    be_t = nc.dram_tensor("be", [1, G4], f32, kind="ExternalInput")
    bd_t = nc.dram_tensor("bd", [1, G4], f32, kind="ExternalInput")

    logits = nc.dram_tensor("logits", [RT, VSH], f32, kind="ExternalOutput")

    dbg = {}
    if debug:
        dbg["memT"] = nc.dram_tensor("dbg_memT", [128, 8, TS, NB], bf, kind="ExternalOutput")
        dbg["c_enc"] = nc.dram_tensor("dbg_cenc", [128, 256], f32, kind="ExternalOutput")
        dbg["keysT"] = nc.dram_tensor("dbg_keysT", [128, 8, NB, TS], bf, kind="ExternalOutput")
        dbg["HallT"] = nc.dram_tensor("dbg_HallT", [128, 8, TD + 1, NB], bf, kind="ExternalOutput")
        dbg["alTall"] = nc.dram_tensor("dbg_alTall", [128, 2, TD, NB], bf, kind="ExternalOutput")
        dbg["MemWca"] = nc.dram_tensor("dbg_MemWca", [128, 2, G4], bf, kind="ExternalOutput")

    with tile.TileContext(nc) as tc, ExitStack() as ctx:
        constp = ctx.enter_context(tc.tile_pool(name="const", bufs=1))
        ident = constp.tile([128, 128], bf)
        make_identity(nc, ident[:])

        dramp = ctx.enter_context(tc.tile_pool(name="dram", bufs=1, space="DRAM"))
        Xe_d = dramp.tile([RE, G4], bf, tag="Xe")
        Xd_d = dramp.tile([RD, G4], bf, tag="Xd")
        aginC = [dramp.tile([8, 128, (c1 - c0) * NB], bf, tag=f"agin{j}",
                            name=f"aginC{j}")
                 for j, (c0, c1) in enumerate(CHUNKS)]
        agoutC = [dramp.tile([NC, 8, 128, (c1 - c0) * NB], bf, tag=f"agout{j}",
                             name=f"agoutC{j}", addr_space="Shared")
                  for j, (c0, c1) in enumerate(CHUNKS)]

        statep = ctx.enter_context(tc.tile_pool(name="state", bufs=1))
        memT = statep.tile([128, 8, TS, NB], bf)       # encoder H^T (= 2h)
        C2 = statep.tile([128, 256], f32)              # 2c (enc then dec)
        keysT = statep.tile([128, 8, NB, TS], bf)      # keys^T, batch-major
        HdecT = statep.tile([128, 8, TD + 1, NB], bf)  # slot t+1 = H_t = 2h_t
        alTall = statep.tile([128, 2, TD, NB], bf)     # block-diag align rows=(q,s), cols=b
        MemWca = statep.tile([128, 2, G4], bf)         # (memT @ Wca'), rows=(q,s)
        MemWaC = statep.tile([128, 2, U], bf)          # (memT @ WaC'), rows=(q,s)

        gp = ctx.enter_context(tc.tile_pool(name="gates", bufs=1))
        xe_pp = [gp.tile([NB, G4], bf, name=f"xe{i}") for i in range(2)]
        tga = gp.tile([128, 512], f32)   # tanh(z_u), tanh(z_i/2)
        tfo = gp.tile([128, 512], f32)   # tanh(z_f/2), tanh(z_o/2)
        IG2 = gp.tile([128, 256], f32)
        FC2 = gp.tile([128, 256], f32)
        tc_t = gp.tile([128, 256], f32)
        Hbf = gp.tile([128, 256], bf)
        h_tr = gp.tile([128, 256], bf, tag="h_tr")

        # ------------- embedding gathers + X precomputes -------------
        def x_precompute_all(jobs):
            with ExitStack() as c2:
                pp = c2.enter_context(tc.tile_pool(name="xpre", bufs=2))
                pp1 = c2.enter_context(tc.tile_pool(name="xpre1", bufs=1))
                psx = c2.enter_context(tc.tile_pool(name="xpre_ps", bufs=1, space="PSUM"))
                tiles = []
                for jj, (idx_t, emb_t, w_t, bias_t, rows, out_d) in enumerate(jobs):
                    nm = (rows + 127) // 128
                    for m in range(nm):
                        r0 = 128 * m
                        rr = min(128 * (m + 1), rows) - r0
                        idx_sb = pp1.tile([128, 1], dt.int32, name=f"idx{jj}_{m}")
                        nc.sync.dma_start(out=idx_sb[:rr, :], in_=idx_t[r0:r0 + rr, :])
                        gath = pp1.tile([128, E], f32, name=f"gath{jj}_{m}")
                        nc.gpsimd.indirect_dma_start(
                            out=gath[:rr, :], out_offset=None,
                            in_=emb_t[:],
                            in_offset=bass.IndirectOffsetOnAxis(ap=idx_sb[:rr, :1],
                                                                axis=0))
                        gbf = pp1.tile([128, E], bf, name=f"gbf{jj}_{m}")
                        nc.vector.tensor_copy(gbf[:rr, :], gath[:rr, :])
                        tiles.append((jj, r0, rr, gbf))
                w_sb = pp1.tile([128, 2, G4], bf, name="wx")
                bias_bc = pp1.tile([128, G4], f32, name="biasbc")
                cur = [None]

                def _stage_wb(jj):
                    w_t, bias_t = jobs[jj][2], jobs[jj][3]
                    for kk in range(2):
                        nc.scalar.dma_start(out=w_sb[:, kk, :],
                                            in_=w_t[128 * kk:128 * (kk + 1), :])
                    nc.scalar.dma_start(out=bias_bc[:],
                                        in_=bias_t[:].to_broadcast([128, G4]))
                    cur[0] = jj

                for jj, r0, rr, gbf in tiles:
                    if cur[0] != jj:
                        _stage_wb(jj)
                    out_d = jobs[jj][5]
                    xT = pp.tile([128, 2, 128], bf, tag="xT")
                    for kk in range(2):
                        pt = psx.tile([128, 128], bf, tag="ptr")
                        nc.tensor.transpose(pt[:, :rr], gbf[:rr, 128 * kk:128 * (kk + 1)],
                                            ident[:rr, :rr])
                        nc.vector.tensor_copy(xT[:, kk, :rr], pt[:, :rr])
                    for chv in range(8):
                        cs0 = 512 * chv
                        ps = psx.tile([128, 512], f32, tag="pmm")
                        for kk in range(2):
                            nc.tensor.matmul(ps[:rr, :], xT[:, kk, :rr],
                                             w_sb[:, kk, cs0:cs0 + 512],
                                             start=(kk == 0), stop=(kk == 1))
                        st = pp.tile([128, 512], bf, tag="stage")
                        nc.vector.tensor_add(st[:rr, :], ps[:rr, :],
                                             bias_bc[:rr, cs0:cs0 + 512])
                        nc.sync.dma_start(out=out_d[r0:r0 + rr, cs0:cs0 + 512],
                                          in_=st[:rr, :])

        def gate_tail(ps, dst_of_h):
            # z in psum ps [128, 1024]; writes H^T into dst_of_h(h) [128, 4, NB]
            # for kk half h, updates C2 in place.  Split into fc-halves so the
            # first half of H^T (kk 0..3) lands early and the next z-stream
            # restarts sooner.
            ps4 = ps[:].rearrange("p (g c) -> p g c", g=4)
            tga4 = tga[:].rearrange("p (g c) -> p g c", g=2)
            tfo4 = tfo[:].rearrange("p (g c) -> p g c", g=2)
            for h in range(2):
                cl, ch = 128 * h, 128 * h + 128
                nc.scalar.activation(tga4[:, :, cl:ch], ps4[:, 0:2, cl:ch],
                                     AF.Tanh)
                nc.scalar.activation(tfo4[:, :, cl:ch], ps4[:, 2:4, cl:ch],
                                     AF.Tanh)
                nc.vector.scalar_tensor_tensor(
                    IG2[:, cl:ch], tga[:, 256 + cl:256 + ch], 1.0,
                    tga[:, cl:ch], op0=ALU.add, op1=ALU.mult)
                nc.vector.scalar_tensor_tensor(
                    FC2[:, cl:ch], tfo[:, cl:ch], 1.0,
                    C2[:, cl:ch], op0=ALU.add, op1=ALU.mult)
                nc.vector.scalar_tensor_tensor(
                    C2[:, cl:ch], FC2[:, cl:ch], 0.5,
                    IG2[:, cl:ch], op0=ALU.mult, op1=ALU.add)
                nc.scalar.activation(tc_t[:, cl:ch], C2[:, cl:ch],
                                     AF.Tanh, scale=0.5)
                nc.vector.scalar_tensor_tensor(
                    Hbf[:, cl:ch], tfo[:, 256 + cl:256 + ch], 1.0,
                    tc_t[:, cl:ch], op0=ALU.add, op1=ALU.mult)
                nc.vector.transpose(h_tr[:, cl:ch], Hbf[:, cl:ch])
                nc.vector.tensor_copy(
                    dst_of_h(h),
                    h_tr[:, cl:ch].rearrange("p (k c) -> p k c", k=4)[:, :, 0:NB])

        # gathered attention activations: scattered per AllGather chunk during
        # the decoder, consumed by the projection after the scan scope closes.
        # Must sit below the scan pools in the pool stack.
        sbagp = ctx.enter_context(tc.tile_pool(name="sbag", bufs=1))
        sb_ag = sbagp.tile([128, NC, 8, TD, NB], bf)

        # ------------- scans (shared psum pool) -------------
        with ExitStack() as scn:
            psp = scn.enter_context(tc.tile_pool(name="scanps", bufs=1, space="PSUM"))
            psum_z0 = psp.tile([128, 1024], f32, tag="pz0")
            psum_z1 = psp.tile([128, 1024], f32, tag="pz1")
            psum_zp = [psum_z0, psum_z1]
            psum_sc = psp.tile([128, 256], f32, tag="psc")
            psum_mw = psp.tile([128, 512], f32, tag="pmw")

            def emit_ids(ps, xe, close):
                # identity matmuls fold the x projection into psum (group start)
                for m in range(4):
                    for chv in range(2):
                        co = 1024 * m + 512 * chv
                        nc.tensor.matmul(
                            ps[32 * m:32 * m + NB, 512 * chv:512 * chv + 512],
                            ident[0:NB, 0:NB], xe[0:NB, co:co + 512],
                            start=True, stop=close,
                            tile_position=(0, 32 * m))

            def emit_z_stream(ps, lhsT_of_kk, w_sb_of_kk, with_align,
                              al_t=None):
                # kk-outer, chv-inner: per col group the two chv matmuls share
                # one stationary load (bass skips the redundant LDWEIGHTS)
                for kk in range(8):
                    lh = lhsT_of_kk(kk)
                    for m in range(4):
                        for chv in range(2):
                            co = 1024 * m + 512 * chv
                            nc.tensor.matmul(
                                ps[32 * m:32 * m + NB, 512 * chv:512 * chv + 512],
                                lh, w_sb_of_kk(kk)[:, co:co + 512],
                                start=False,
                                stop=(kk == 7 and not with_align),
                                tile_position=(0, 32 * m))
                if with_align:
                    for m in range(4):
                        for p in range(2):
                            for chv in range(2):
                                co = 1024 * m + 512 * chv
                                nc.tensor.matmul(
                                    ps[32 * m:32 * m + NB, 512 * chv:512 * chv + 512],
                                    alTall[:, p, al_t, :],
                                    MemWca[:, p, co:co + 512],
                                    start=False, stop=(p == 1),
                                    tile_position=(0, 32 * m))

            # x precompute first: its staging pools need the space the big
            # weight pools occupy later.
            x_precompute_all([
                (enc_idx, enc_emb, Wxe, be_t, RE, Xe_d),
                (dec_idx, dec_emb, Wxdx, bd_t, RD, Xd_d),
            ])

            # Whcomb: 6 chunks prefetched during the encoder (gpsimd queue
            # is idle); the last 2 chunks load once Wca's space frees up.
            whcp = scn.enter_context(tc.tile_pool(name="whc", bufs=1))
            whc_a = whcp.tile([128, 6, G4], bf)
            whc_b = None

            def whc_of_kk(kk):
                return whc_a[:, kk, :] if kk < 6 else whc_b[:, kk - 6, :]

            # ---------------- encoder ----------------
            with ExitStack() as ec:
                encp = ec.enter_context(tc.tile_pool(name="enc", bufs=1))
                whe_sb = encp.tile([128, 8, G4], bf)
                for kk in range(8):
                    nc.scalar.dma_start(out=whe_sb[:, kk, :],
                                        in_=Whe[128 * kk:128 * (kk + 1), :])

                nc.vector.memset(C2[:], 0.0)

                nc.sync.dma_start(out=xe_pp[0][:], in_=Xe_d[0:NB, :])
                emit_ids(psum_zp[0], xe_pp[0], close=True)
                for t in range(TS):
                    ps = psum_zp[t % 2]
                    if t + 1 < TS:
                        nc.sync.dma_start(out=xe_pp[(t + 1) % 2][:],
                                          in_=Xe_d[NB * (t + 1):NB * (t + 2), :])
                    if t > 0:
                        emit_z_stream(ps,
                                      (lambda kk, _t=t: memT[:, kk, _t - 1, :]),
                                      (lambda kk: whe_sb[:, kk, :]),
                                      with_align=False)
                    # next step's id matmuls go in front of the tail so they
                    # fill the PE gap (they only need the x tile)
                    if t + 1 < TS:
                        emit_ids(psum_zp[(t + 1) % 2], xe_pp[(t + 1) % 2],
                                 close=False)
                    gate_tail(ps, (lambda h, _t=t:
                                   memT[:, 4 * h:4 * h + 4, _t, :]))
                    # prefetch most of Whcomb on the idle gpsimd queue
                    # (last 2 chunks wait for the Wca space at the transition)
                    if stage != "enc" and t == 40:
                        for kk in range(6):
                            nc.gpsimd.dma_start(
                                out=whc_a[:, kk, :],
                                in_=Whcomb[128 * kk:128 * (kk + 1), :])

                if debug:
                    nc.sync.dma_start(out=dbg["memT"][:], in_=memT[:])
                    nc.sync.dma_start(out=dbg["c_enc"][:], in_=C2[:])

            # ---------------- transition: keys, MemWca, MemWaC ----------------
            m_dec = _re.match(r"dec(\d+)$", stage)
            TD_RUN = int(m_dec.group(1)) if m_dec else TD
            if stage != "enc":
                decp = scn.enter_context(tc.tile_pool(name="dec", bufs=1))

                memQ = decp.tile([128, 8, 2, 128], bf)

                with ExitStack() as c3:
                    wmp = c3.enter_context(tc.tile_pool(name="wmp", bufs=1))
                    wm_sb = wmp.tile([128, 8, U], bf)
                    wm_src = Wm_t[:].rearrange("(k p) c -> p k c", k=8)
                    for ko in range(8):
                        nc.gpsimd.dma_start(
                            out=wm_sb[:, :, 128 * ko:128 * (ko + 1)],
                            in_=wm_src[:, :, 128 * ko:128 * (ko + 1)])
                    # keysT = (memT @ Wm')^T, stored batch-major [p, kk, b, s].
                    # 4 rotating psum accumulators (z banks are idle here) keep
                    # independent chains in flight so LDWEIGHTS stays hidden.
                    rot = [psum_mw[:, 0:256], psum_z0[:, 0:256],
                           psum_z0[:, 512:768], psum_z1[:, 0:256]]
                    for ko in range(8):
                        pa = rot[ko % 4]
                        for kk in range(8):
                            nc.tensor.matmul(pa,
                                             wm_sb[:, kk, 128 * ko:128 * (ko + 1)],
                                             memT[:, kk, :, :],
                                             start=(kk == 0), stop=(kk == 7))
                        nc.vector.tensor_copy(
                            keysT[:, ko],
                            pa.rearrange("p (s b) -> p b s", b=NB))

                    # memQ[:, kk, p, 64q+s] = memT[:, kk, s, 2p+q]
                    for kk in range(8):
                        for p in range(2):
                            nc.vector.tensor_copy(
                                memQ[:, kk, p, :].rearrange("p (q s) -> p q s", q=2),
                                memT[:, kk, :, 2 * p:2 * p + 2].rearrange(
                                    "p s q -> p q s"))

                with ExitStack() as c3b:
                    wcap2 = c3b.enter_context(tc.tile_pool(name="wca2", bufs=1))
                    wca_sb = wcap2.tile([128, 8, G4], bf)
                    # column-block-major: the first MemWca chain (c8=0) only
                    # needs cols 0:512 of every kk chunk, so it starts after
                    # ~1MB instead of the full 8MB load
                    wca_src = Wca_t[:].rearrange("(k p) c -> p k c", k=8)
                    for c8 in range(8):
                        nc.gpsimd.dma_start(
                            out=wca_sb[:, :, 512 * c8:512 * (c8 + 1)],
                            in_=wca_src[:, :, 512 * c8:512 * (c8 + 1)])
                    rot2 = [psum_mw[:], psum_z0[:, 0:512],
                            psum_z0[:, 512:1024], psum_z1[:, 0:512]]
                    for c8 in range(8):
                        for p in range(2):
                            pa = rot2[(2 * c8 + p) % 4]
                            for kk in range(8):
                                nc.tensor.matmul(
                                    pa, memQ[:, kk, p, :],
                                    wca_sb[:, kk, 512 * c8:512 * (c8 + 1)],
                                    start=(kk == 0), stop=(kk == 7))
                            nc.vector.tensor_copy(
                                MemWca[:, p, 512 * c8:512 * (c8 + 1)], pa)

                whcp2 = scn.enter_context(tc.tile_pool(name="whc2", bufs=1))
                whc_b = whcp2.tile([128, 2, G4], bf)
                # decoder x tiles can load as soon as the encoder stops
                # touching the ping-pong buffers
                nc.sync.dma_start(out=xe_pp[0][:], in_=Xd_d[0:NB, :])
                nc.sync.dma_start(out=xe_pp[1][:], in_=Xd_d[NB:2 * NB, :])

                with ExitStack() as c3c:
                    wacp = c3c.enter_context(tc.tile_pool(name="wacp", bufs=1))
                    wac_sb = wacp.tile([128, 8, U], bf)
                    for kk in range(8):
                        nc.gpsimd.dma_start(out=wac_sb[:, kk, :],
                                            in_=WaC_t[128 * kk:128 * (kk + 1), :])
                    rot3 = [psum_mw[:], psum_z0[:, 0:512],
                            psum_z0[:, 512:1024], psum_z1[:, 0:512]]
                    for p in range(2):
                        for c2_ in range(2):
                            pa = rot3[(2 * p + c2_) % 4]
                            for kk in range(8):
                                nc.tensor.matmul(
                                    pa, memQ[:, kk, p, :],
                                    wac_sb[:, kk, 512 * c2_:512 * (c2_ + 1)],
                                    start=(kk == 0), stop=(kk == 7))
                            nc.vector.tensor_copy(
                                MemWaC[:, p, 512 * c2_:512 * (c2_ + 1)], pa)

                if debug:
                    nc.sync.dma_start(out=dbg["keysT"][:], in_=keysT[:])
                    nc.sync.dma_start(out=dbg["MemWca"][:], in_=MemWca[:])

                # ---------------- decoder scan ----------------
                nc.vector.memset(alTall[:], 0.0)

                rsums = decp.tile([128, NB], f32)
                rmask = decp.tile([128, NB], f32)
                rsD = decp.tile([128, 1], f32)
                # rmask[96+p, b] = 1 iff p == b (diag selector)
                nc.vector.tensor_copy(rmask[96:128, :], ident[96:128, 96:96 + NB])

                exp_sc = None
                align_bf = None
                dve_t = None
                attnT = None
                wah_sb = None

                def softmax_emit(t):
                    # scores in psum_sc rows 96:100 -> alTall[:, :, t, :]
                    nc.scalar.activation(exp_sc[96:128, :], psum_sc[96:128, :], AF.Exp)
                    for b in range(NB):
                        nc.vector.reduce_sum(rsums[96:128, b:b + 1],
                                             exp_sc[96:128, 64 * b:64 * (b + 1)],
                                             axis=AX.X)
                    nc.vector.tensor_mul(rsums[96:128, :], rsums[96:128, :],
                                         rmask[96:128, :])
                    nc.vector.reduce_sum(rsD[96:128, :], rsums[96:128, :], axis=AX.X)
                    nc.vector.reciprocal(rsD[96:128, :], rsD[96:128, :])
                    nc.vector.tensor_scalar(align_bf[96:128, :], exp_sc[96:128, :],
                                            rsD[96:128, 0:1], None, op0=ALU.mult)
                    nc.vector.transpose(dve_t[96:128, :], align_bf[96:128, :])
                    # diag value align_b[32h+r] sits at dve_t[96+r, 32*(2b+h)+b]
                    for b in range(NB):
                        p, q = b // 2, b % 2
                        for hh in range(2):
                            cc = 32 * (2 * b + hh) + b
                            nc.vector.tensor_copy(
                                alTall[64 * q + 32 * hh:64 * q + 32 * hh + 32,
                                       p, t, b:b + 1],
                                dve_t[96:128, cc:cc + 1])

                def attn_chunk_mms(j, kos):
                    c0, c1 = CHUNKS[j]
                    cw = (c1 - c0) * NB
                    for ko in kos:
                        pa = psum_mw[:, 0:cw]
                        for kk in range(8):
                            nc.tensor.matmul(
                                pa, wah_sb[:, kk, 128 * ko:128 * (ko + 1)],
                                HdecT[:, kk, 1 + c0:1 + c1, :],
                                start=(kk == 0), stop=False)
                        for p in range(2):
                            nc.tensor.matmul(
                                pa,
                                MemWaC[:, p, 128 * ko:128 * (ko + 1)],
                                alTall[:, p, c0:c1, :].rearrange(
                                    "p t b -> p (t b)"),
                                start=False, stop=(p == 1))
                        nc.vector.tensor_copy(attnT[:, ko, 0:cw], pa)

                def attn_chunk_fin(j):
                    c0, c1 = CHUNKS[j]
                    cw = (c1 - c0) * NB
                    nc.gpsimd.dma_start(
                        out=aginC[j][:].rearrange("k p c -> p k c"),
                        in_=attnT[:, :, 0:cw])
                    nc.gpsimd.collective_compute(
                        "AllGather", ALU.bypass,
                        ins=[aginC[j][:]], outs=[agoutC[j][:]],
                        replica_groups=[list(range(NC))])
                    for r in range(NC):
                        nc.gpsimd.dma_start(
                            out=sb_ag[:, r, :, c0:c1, :],
                            in_=agoutC[j][r].rearrange("k p (t b) -> p k t b",
                                                       b=NB))

                def attn_chunk(j):
                    attn_chunk_mms(j, range(8))
                    attn_chunk_fin(j)

                # chunks 0-2 spread their matmul bursts over 3 steps' tail
                # gaps; the last two chunks stay immediate (tail-critical)
                attn_sched = {}
                for j, (c0, c1) in enumerate(CHUNKS[:3]):
                    attn_sched[c1 - 1] = (j, [0, 1, 2])
                    attn_sched[c1] = (j, [3, 4, 5])
                    attn_sched[c1 + 1] = (j, [6, 7], 'fin')

                def scores_emit(t):
                    for kk in range(8):
                        nc.tensor.matmul(
                            psum_sc[96:96 + NB, :],
                            HdecT[:, kk, t + 1, :],
                            keysT[:, kk].rearrange("p b s -> p (b s)"),
                            start=(kk == 0), stop=(kk == 7),
                            tile_position=(0, 96))

                # streamed t=0 weights (Whd0) in a scoped pool
                with ExitStack() as c4:
                    w0p = c4.enter_context(tc.tile_pool(name="w0", bufs=3))
                    w0_tiles = []
                    for kk in range(8):
                        w0 = w0p.tile([128, G4], bf, tag="w0")
                        nc.gpsimd.dma_start(out=w0[:],
                                            in_=Whd0[128 * kk:128 * (kk + 1), :])
                        w0_tiles.append(w0)
                    # less-urgent weights load after the z_0 stream weights
                    for kk in range(6, 8):
                        nc.gpsimd.dma_start(
                            out=whc_b[:, kk - 6, :],
                            in_=Whcomb[128 * kk:128 * (kk + 1), :])

                    ps = psum_zp[0]
                    emit_ids(ps, xe_pp[0], close=False)
                    emit_z_stream(ps,
                                  (lambda kk: memT[:, kk, TS - 1, :]),
                                  (lambda kk: w0_tiles[kk]),
                                  with_align=False)
                    emit_ids(psum_zp[1], xe_pp[1], close=False)
                    gate_tail(ps, (lambda h: HdecT[:, 4 * h:4 * h + 4, 1, :]))
                    scores_emit(0)

                # softmax scratch + attn staging + WaH + gathered activations
                # (allocated after the w0 pool frees its space)
                dec2p = scn.enter_context(tc.tile_pool(name="dec2", bufs=1))
                exp_sc = dec2p.tile([128, 256], f32)
                align_bf = dec2p.tile([128, 256], bf)
                dve_t = dec2p.tile([128, 256], bf)
                attnT = dec2p.tile([128, 8, 64], bf)   # per-chunk staging
                wah_sb = dec2p.tile([128, 8, U], bf)
                # wah rides the idle gpsimd queue; needed first at t=15
                for kk in range(8):
                    nc.gpsimd.dma_start(out=wah_sb[:, kk, :],
                                        in_=WaH_t[128 * kk:128 * (kk + 1), :])
                softmax_emit(0)

                for t in range(1, TD_RUN):
                    ps = psum_zp[t % 2]
                    if t + 1 < TD_RUN:
                        nc.sync.dma_start(out=xe_pp[(t + 1) % 2][:],
                                          in_=Xd_d[NB * (t + 1):NB * (t + 2), :])
                    emit_z_stream(ps,
                                  (lambda kk, _t=t: HdecT[:, kk, _t, :]),
                                  whc_of_kk,
                                  with_align=True, al_t=t - 1)
                    if t + 1 < TD_RUN:
                        emit_ids(psum_zp[(t + 1) % 2], xe_pp[(t + 1) % 2],
                                 close=False)
                    gate_tail(ps, (lambda h, _t=t:
                                   HdecT[:, 4 * h:4 * h + 4, _t + 1, :]))
                    scores_emit(t)
                    softmax_emit(t)
                    if stage == "full":
                        if t in attn_sched:
                            ent = attn_sched[t]
                            attn_chunk_mms(ent[0], ent[1])
                            if len(ent) > 2:
                                attn_chunk_fin(ent[0])
                        elif (t + 1) in [c1 for _, c1 in CHUNKS[3:]]:
                            attn_chunk(
                                3 + [c1 for _, c1 in CHUNKS[3:]].index(t + 1))

                if debug:
                    nc.sync.dma_start(out=dbg["HallT"][:], in_=HdecT[:])
                    nc.sync.dma_start(out=dbg["alTall"][:], in_=alTall[:])

        # ------- projection (sb_ag filled by the chunked AllGather) -------
        if stage == "full":
            with ExitStack() as c2:
                ppd = c2.enter_context(tc.tile_pool(name="projd", bufs=4))
                ps4 = c2.enter_context(tc.tile_pool(name="projps", bufs=1,
                                                    space="PSUM"))
                wfp = c2.enter_context(tc.tile_pool(name="wfc", bufs=1))
                # all of Wf resident: one stationary load serves all 8 vocab
                # chunks of a row tile (LDWEIGHTS amortized 8x)
                wf_all = wfp.tile([128, 8, VSH], bf)
                wfs_src = Wfs[:].rearrange("(k p) c -> p k c", k=8)
                for sc in range(8):
                    nc.scalar.dma_start(
                        out=wf_all[:, :, 500 * sc:500 * (sc + 1)],
                        in_=wfs_src[:, :, 500 * sc:500 * (sc + 1)])
                bf_all = wfp.tile([128, VSH], f32)
                nc.scalar.dma_start(out=bf_all[:],
                                    in_=bfs[:].to_broadcast([128, VSH]))
                pj_t = [ps4.tile([128, 500], f32, name=f"pj{i}")
                        for i in range(8)]
                for th in range(2):
                    for r in range(NC):
                        t0 = 32 * th
                        t1 = min(t0 + 32, TD)
                        rr = (t1 - t0) * NB
                        r0 = 252 * r + NB * t0
                        lhs = [sb_ag[:, r, kk, t0:t1, :].rearrange(
                                   "p t b -> p (t b)") for kk in range(8)]
                        # sc-outer: each vocab chunk's accumulation group
                        # closes early so its bias-add and output DMA overlap
                        # the next chunk's matmuls
                        for sc in range(8):
                            for kk in range(8):
                                nc.tensor.matmul(
                                    pj_t[sc][:rr, :], lhs[kk],
                                    wf_all[:, kk, 500 * sc:500 * (sc + 1)],
                                    start=(kk == 0), stop=(kk == 7))
                            st = ppd.tile([128, 500], f32, tag="st")
                            nc.vector.tensor_add(
                                st[:rr, :], pj_t[sc][:rr, :],
                                bf_all[:rr, 500 * sc:500 * (sc + 1)])
                            nc.sync.dma_start(
                                out=logits[r0:r0 + rr,
                                           500 * sc:500 * (sc + 1)],
                                in_=st[:rr, :])

        if stage != "full":
            # partial-stage dummy output so the NEFF has its ExternalOutput written
            st0 = gp.tile([1, 4], f32, tag="dummy")
            nc.vector.tensor_copy(st0[:], tga[0:1, 0:4])
            nc.sync.dma_start(out=logits[0:1, 0:4], in_=st0[:])

    nc.finalize()
    return nc, dbg


_CACHE = {}


def _get_nc(stage="full", debug=False):
    key = (stage, debug)
    if key not in _CACHE:
        _CACHE[key] = _build_nc(stage, debug)
    return _CACHE[key]


def run_cores(inputs, stage="full", debug=False, trace=False):
    from concourse.bass_utils import run_bass_kernel_spmd
    shared, per_core = _prep_host(inputs)
    nc, dbg = _get_nc(stage, debug)
    in_maps = []
    for k in range(NC):
        m = dict(shared)
        m.update(per_core[k])
        in_maps.append(m)
    return run_bass_kernel_spmd(nc, in_maps, core_ids=list(range(NC)), trace=trace)


def unshard(outs):
    full = np.concatenate(outs, axis=1)                     # [2016, 32000]
    # rows ordered (r, t, b_local); batch b = 4*r + b_local
    full = full.reshape(NC, TD, NB, VT).transpose(0, 2, 1, 3).reshape(B, TD, VT)
    return np.ascontiguousarray(full.astype(np.float32))


def kernel(**inputs):
    res = run_cores(inputs, stage="full")
    outs = [np.asarray(r["logits"]) for r in res.results]   # [2016, 4000] each
    return unshard(outs)


# revision 46
# speedup vs baseline: 1.0087x; 1.0087x over previous
"""Trainium2 Bass kernel for nn_DmTranslateTrain (seq2seq translate train step).

Strategy (8 NeuronCores, SPMD):
  - Data-parallel over batch: core k owns batches [4k, 4k+4). Each core runs the
    full encoder LSTM scan + decoder (LSTM + Luong attention) for its 4 batches.
  - Output projection is tensor-parallel over the vocabulary: chunked AllGather
    of attention activations overlapped with the decoder, then each core
    computes logits[:, 4000k:4000k+4000].

Scan-step design (the hot loop):
  - The x-projection (emb @ Wx + b, precomputed in DRAM) is folded into the PE
    accumulation with a tiny K=4 identity matmul, so the gate nonlinearities
    read PSUM directly (no vector adds on the critical path).
  - All four gates use plain tanh: sigma(x) = (1+tanh(x/2))/2, with the 0.5
    pre-scale for gates i/f/o folded into the weight columns host-side.  One
    fused tanh per 512-col gate pair (2 ACTs per step), one activation table.
  - State kept scaled: C2 = 2c, H = 2h.  Updates via scalar_tensor_tensor:
      IG2 = (ti+1)*tu; FC2 = (tf+1)*C2; C2' = 0.5*FC2 + IG2;
      tc = tanh(0.5*C2'); H = (to+1)*tc.
  - z matmuls emitted kk-outer / m-inner so the 4 PE column-groups
    (tile_position=(0,32m)) stream concurrently; decoder scores run in column
    group q96 on separate PSUM partitions.

Gate packing: z tile is [128, 1024] per band m (partition = 32*m + b), free
col = gate*256 + 32*fc + r for unit u = 128*fc + 32*m + r, gates ordered
[u, i, f, o] (u = candidate).  The DVE 32x32 block transpose of the H tile
directly yields H^T in natural u-major chunks (one copy per step).
Logits rows are ordered (core, t, local batch); the host unshards.
"""

import numpy as np

B, TS, TD = 32, 64, 63
VS, VT = 32000, 32000
E, U = 256, 1024
G4 = 4 * U
NB = 4            # batches per core
NC = 8            # cores
VSH = VT // NC    # vocab shard per core
RE = TS * NB      # encoder rows per core
RD = TD * NB      # decoder rows per core
RT = TD * B       # total decoder rows (all batches)

_GATE_PERM = [2, 0, 1, 3]  # new order [u, i, f, o] -> original gate index
CHUNKS = [(0, 16), (16, 32), (32, 48), (48, 60), (60, TD)]


def _reorder_cols(w):
    # natural col = gate_orig*1024 + u, u = 128*fc + 32*m + r
    w5 = w.reshape(w.shape[0], 4, 8, 4, 32)        # [in, g_orig, fc, m, r]
    w5 = w5[:, _GATE_PERM]                          # [in, g_new, fc, m, r]
    w5 = w5.transpose(0, 3, 1, 2, 4)                # [in, m, g_new, fc, r]
    return np.ascontiguousarray(w5.reshape(w.shape[0], G4))


def _reorder_bias(b):
    b5 = b.reshape(4, 8, 4, 32)[_GATE_PERM].transpose(2, 0, 1, 3)
    return np.ascontiguousarray(b5.reshape(1, G4))


def _prep_host(inputs):
    import ml_dtypes
    bf16 = ml_dtypes.bfloat16
    f32 = np.float32
    enc_in = np.asarray(inputs["encoder_input"])
    dec_in = np.asarray(inputs["decoder_input"])
    Wx_e = np.asarray(inputs["Wx_e"], f32)
    Wh_e = np.asarray(inputs["Wh_e"], f32)
    b_e = np.asarray(inputs["b_e"], f32)
    Wx_d = np.asarray(inputs["Wx_d"], f32)
    Wh_d = np.asarray(inputs["Wh_d"], f32)
    b_d = np.asarray(inputs["b_d"], f32)
    Wm = np.asarray(inputs["Wm"], f32)
    Wa = np.asarray(inputs["Wa"], f32)
    Wf = np.asarray(inputs["Wf"], f32)
    bfv = np.asarray(inputs["bf"], f32)

    Wxd_x = Wx_d[:E]
    Wxd_a = Wx_d[E:]
    Wa_h, Wa_c = Wa[:U], Wa[U:]

    # per-gate column scale on the NATURAL layout (i, f, g, o): tanh trick
    # needs 0.5*z for i/f/o; the candidate gate g keeps full scale.
    cs = np.concatenate([np.full(U, 0.5, f32), np.full(U, 0.5, f32),
                         np.ones(U, f32), np.full(U, 0.5, f32)])

    shared = {
        "Wxe": _reorder_cols(Wx_e * cs).astype(bf16),
        "Whe": _reorder_cols(0.5 * Wh_e * cs).astype(bf16),
        "Whcomb": _reorder_cols(0.5 * (Wh_d + Wa_h @ Wxd_a) * cs).astype(bf16),
        "Wca": _reorder_cols(0.5 * (Wa_c @ Wxd_a) * cs).astype(bf16),
        "Whd0": _reorder_cols(0.5 * Wh_d * cs).astype(bf16),
        "Wxdx": _reorder_cols(Wxd_x * cs).astype(bf16),
        "Wm": (0.25 * Wm).astype(bf16),
        "WaH": (0.5 * Wa_h).astype(bf16),
        "WaC": np.ascontiguousarray((0.5 * Wa_c).astype(bf16)),
        "be": _reorder_bias(b_e * cs),
        "bd": _reorder_bias(b_d * cs),
        "enc_emb": np.ascontiguousarray(np.asarray(inputs["enc_emb"], f32)),
        "dec_emb": np.ascontiguousarray(np.asarray(inputs["dec_emb"], f32)),
    }
    Wf_bf = Wf.astype(bf16)
    per_core = []
    for k in range(NC):
        eidx = enc_in[NB * k:NB * (k + 1)]
        didx = dec_in[NB * k:NB * (k + 1)]
        per_core.append({
            "enc_idx": np.ascontiguousarray(eidx.T.reshape(RE, 1).astype(np.int32)),
            "dec_idx": np.ascontiguousarray(didx.T.reshape(RD, 1).astype(np.int32)),
            "Wfs": np.ascontiguousarray(Wf_bf[:, VSH * k:VSH * (k + 1)]),
            "bfs": np.ascontiguousarray(bfv[VSH * k:VSH * (k + 1)].reshape(1, VSH)),
        })
    return shared, per_core


# ---------------------------------------------------------------------------

def _build_nc(stage="full", debug=False):
    import re as _re
    from contextlib import ExitStack
    import concourse.bass as bass
    import concourse.mybir as mybir
    import concourse.tile as tile
    from concourse import bacc
    from concourse.masks import make_identity

    dt = mybir.dt
    AF = mybir.ActivationFunctionType
    ALU = mybir.AluOpType
    AX = mybir.AxisListType
    f32, bf = dt.float32, dt.bfloat16

    nc = bacc.Bacc("TRN2", target_bir_lowering=False, debug=False, num_devices=NC)

    enc_idx = nc.dram_tensor("enc_idx", [RE, 1], dt.int32, kind="ExternalInput")
    dec_idx = nc.dram_tensor("dec_idx", [RD, 1], dt.int32, kind="ExternalInput")
    enc_emb = nc.dram_tensor("enc_emb", [VS, E], f32, kind="ExternalInput")
    dec_emb = nc.dram_tensor("dec_emb", [VT, E], f32, kind="ExternalInput")
    Wxe = nc.dram_tensor("Wxe", [E, G4], bf, kind="ExternalInput")
    Whe = nc.dram_tensor("Whe", [U, G4], bf, kind="ExternalInput")
    Whcomb = nc.dram_tensor("Whcomb", [U, G4], bf, kind="ExternalInput")
    Wca_t = nc.dram_tensor("Wca", [U, G4], bf, kind="ExternalInput")
    Whd0 = nc.dram_tensor("Whd0", [U, G4], bf, kind="ExternalInput")
    Wxdx = nc.dram_tensor("Wxdx", [E, G4], bf, kind="ExternalInput")
    Wm_t = nc.dram_tensor("Wm", [U, U], bf, kind="ExternalInput")
    WaH_t = nc.dram_tensor("WaH", [U, U], bf, kind="ExternalInput")
    WaC_t = nc.dram_tensor("WaC", [U, U], bf, kind="ExternalInput")
    Wfs = nc.dram_tensor("Wfs", [U, VSH], bf, kind="ExternalInput")
    bfs = nc.dram_tensor("bfs", [1, VSH], f32, kind="ExternalInput")
    be_t = nc.dram_tensor("be", [1, G4], f32, kind="ExternalInput")
    bd_t = nc.dram_tensor("bd", [1, G4], f32, kind="ExternalInput")

    logits = nc.dram_tensor("logits", [RT, VSH], f32, kind="ExternalOutput")

    dbg = {}
    if debug:
        dbg["memT"] = nc.dram_tensor("dbg_memT", [128, 8, TS, NB], bf, kind="ExternalOutput")
        dbg["c_enc"] = nc.dram_tensor("dbg_cenc", [128, 256], f32, kind="ExternalOutput")
        dbg["keysT"] = nc.dram_tensor("dbg_keysT", [128, 8, NB, TS], bf, kind="ExternalOutput")
        dbg["HallT"] = nc.dram_tensor("dbg_HallT", [128, 8, TD + 1, NB], bf, kind="ExternalOutput")
        dbg["alTall"] = nc.dram_tensor("dbg_alTall", [128, 2, TD, NB], bf, kind="ExternalOutput")
        dbg["MemWca"] = nc.dram_tensor("dbg_MemWca", [128, 2, G4], bf, kind="ExternalOutput")

    with tile.TileContext(nc) as tc, ExitStack() as ctx:
        constp = ctx.enter_context(tc.tile_pool(name="const", bufs=1))
        ident = constp.tile([128, 128], bf)
        make_identity(nc, ident[:])

        dramp = ctx.enter_context(tc.tile_pool(name="dram", bufs=1, space="DRAM"))
        Xe_d = dramp.tile([RE, G4], bf, tag="Xe")
        Xd_d = dramp.tile([RD, G4], bf, tag="Xd")
        aginC = [dramp.tile([8, 128, (c1 - c0) * NB], bf, tag=f"agin{j}",
                            name=f"aginC{j}")
                 for j, (c0, c1) in enumerate(CHUNKS)]
        agoutC = [dramp.tile([NC, 8, 128, (c1 - c0) * NB], bf, tag=f"agout{j}",
                             name=f"agoutC{j}", addr_space="Shared")
                  for j, (c0, c1) in enumerate(CHUNKS)]

        statep = ctx.enter_context(tc.tile_pool(name="state", bufs=1))
        memT = statep.tile([128, 8, TS, NB], bf)       # encoder H^T (= 2h)
        C2 = statep.tile([128, 256], f32)              # 2c (enc then dec)
        keysT = statep.tile([128, 8, NB, TS], bf)      # keys^T, batch-major
        HdecT = statep.tile([128, 8, TD + 1, NB], bf)  # slot t+1 = H_t = 2h_t
        alTall = statep.tile([128, 2, TD, NB], bf)     # block-diag align rows=(q,s), cols=b
        MemWca = statep.tile([128, 2, G4], bf)         # (memT @ Wca'), rows=(q,s)
        MemWaC = statep.tile([128, 2, U], bf)          # (memT @ WaC'), rows=(q,s)

        gp = ctx.enter_context(tc.tile_pool(name="gates", bufs=1))
        xe_pp = [gp.tile([NB, G4], bf, name=f"xe{i}") for i in range(2)]
        tga = gp.tile([128, 512], f32)   # tanh(z_u), tanh(z_i/2)
        tfo = gp.tile([128, 512], f32)   # tanh(z_f/2), tanh(z_o/2)
        IG2 = gp.tile([128, 256], f32)
        FC2 = gp.tile([128, 256], f32)
        tc_t = gp.tile([128, 256], f32)
        Hbf = gp.tile([128, 256], bf)
        h_tr = gp.tile([128, 256], bf, tag="h_tr")

        # ------------- embedding gathers + X precomputes -------------
        def x_precompute_all(jobs):
            with ExitStack() as c2:
                pp = c2.enter_context(tc.tile_pool(name="xpre", bufs=2))
                pp1 = c2.enter_context(tc.tile_pool(name="xpre1", bufs=1))
                psx = c2.enter_context(tc.tile_pool(name="xpre_ps", bufs=1, space="PSUM"))
                tiles = []
                for jj, (idx_t, emb_t, w_t, bias_t, rows, out_d) in enumerate(jobs):
                    nm = (rows + 127) // 128
                    for m in range(nm):
                        r0 = 128 * m
                        rr = min(128 * (m + 1), rows) - r0
                        idx_sb = pp1.tile([128, 1], dt.int32, name=f"idx{jj}_{m}")
                        nc.sync.dma_start(out=idx_sb[:rr, :], in_=idx_t[r0:r0 + rr, :])
                        gath = pp1.tile([128, E], f32, name=f"gath{jj}_{m}")
                        nc.gpsimd.indirect_dma_start(
                            out=gath[:rr, :], out_offset=None,
                            in_=emb_t[:],
                            in_offset=bass.IndirectOffsetOnAxis(ap=idx_sb[:rr, :1],
                                                                axis=0))
                        gbf = pp1.tile([128, E], bf, name=f"gbf{jj}_{m}")
                        nc.vector.tensor_copy(gbf[:rr, :], gath[:rr, :])
                        tiles.append((jj, r0, rr, gbf))
                w_sb = pp1.tile([128, 2, G4], bf, name="wx")
                bias_bc = pp1.tile([128, G4], f32, name="biasbc")
                cur = [None]

                def _stage_wb(jj):
                    w_t, bias_t = jobs[jj][2], jobs[jj][3]
                    for kk in range(2):
                        nc.scalar.dma_start(out=w_sb[:, kk, :],
                                            in_=w_t[128 * kk:128 * (kk + 1), :])
                    nc.scalar.dma_start(out=bias_bc[:],
                                        in_=bias_t[:].to_broadcast([128, G4]))
                    cur[0] = jj

                for jj, r0, rr, gbf in tiles:
                    if cur[0] != jj:
                        _stage_wb(jj)
                    out_d = jobs[jj][5]
                    xT = pp.tile([128, 2, 128], bf, tag="xT")
                    for kk in range(2):
                        pt = psx.tile([128, 128], bf, tag="ptr")
                        nc.tensor.transpose(pt[:, :rr], gbf[:rr, 128 * kk:128 * (kk + 1)],
                                            ident[:rr, :rr])
                        nc.vector.tensor_copy(xT[:, kk, :rr], pt[:, :rr])
                    for chv in range(8):
                        cs0 = 512 * chv
                        ps = psx.tile([128, 512], f32, tag="pmm")
                        for kk in range(2):
                            nc.tensor.matmul(ps[:rr, :], xT[:, kk, :rr],
                                             w_sb[:, kk, cs0:cs0 + 512],
                                             start=(kk == 0), stop=(kk == 1))
                        st = pp.tile([128, 512], bf, tag="stage")
                        nc.vector.tensor_add(st[:rr, :], ps[:rr, :],
                                             bias_bc[:rr, cs0:cs0 + 512])
                        nc.sync.dma_start(out=out_d[r0:r0 + rr, cs0:cs0 + 512],
                                          in_=st[:rr, :])

        def gate_tail(ps, dst_of_h):
            # z in psum ps [128, 1024]; writes H^T into dst_of_h(h) [128, 4, NB]
            # for kk half h, updates C2 in place.  Split into fc-halves so the
            # first half of H^T (kk 0..3) lands early and the next z-stream
            # restarts sooner.
            ps4 = ps[:].rearrange("p (g c) -> p g c", g=4)
            tga4 = tga[:].rearrange("p (g c) -> p g c", g=2)
            tfo4 = tfo[:].rearrange("p (g c) -> p g c", g=2)
            for h in range(2):
                cl, ch = 128 * h, 128 * h + 128
                nc.scalar.activation(tga4[:, :, cl:ch], ps4[:, 0:2, cl:ch],
                                     AF.Tanh)
                nc.scalar.activation(tfo4[:, :, cl:ch], ps4[:, 2:4, cl:ch],
                                     AF.Tanh)
                nc.vector.scalar_tensor_tensor(
                    IG2[:, cl:ch], tga[:, 256 + cl:256 + ch], 1.0,
                    tga[:, cl:ch], op0=ALU.add, op1=ALU.mult)
                nc.vector.scalar_tensor_tensor(
                    FC2[:, cl:ch], tfo[:, cl:ch], 1.0,
                    C2[:, cl:ch], op0=ALU.add, op1=ALU.mult)
                nc.vector.scalar_tensor_tensor(
                    C2[:, cl:ch], FC2[:, cl:ch], 0.5,
                    IG2[:, cl:ch], op0=ALU.mult, op1=ALU.add)
                nc.scalar.activation(tc_t[:, cl:ch], C2[:, cl:ch],
                                     AF.Tanh, scale=0.5)
                nc.vector.scalar_tensor_tensor(
                    Hbf[:, cl:ch], tfo[:, 256 + cl:256 + ch], 1.0,
                    tc_t[:, cl:ch], op0=ALU.add, op1=ALU.mult)
                nc.vector.transpose(h_tr[:, cl:ch], Hbf[:, cl:ch])
                nc.vector.tensor_copy(
                    dst_of_h(h),
                    h_tr[:, cl:ch].rearrange("p (k c) -> p k c", k=4)[:, :, 0:NB])

        # gathered attention activations: scattered per AllGather chunk during
        # the decoder, consumed by the projection after the scan scope closes.
        # Must sit below the scan pools in the pool stack.
        sbagp = ctx.enter_context(tc.tile_pool(name="sbag", bufs=1))
        sb_ag = sbagp.tile([128, NC, 8, TD, NB], bf)

        # ------------- scans (shared psum pool) -------------
        with ExitStack() as scn:
            psp = scn.enter_context(tc.tile_pool(name="scanps", bufs=1, space="PSUM"))
            psum_z0 = psp.tile([128, 1024], f32, tag="pz0")
            psum_z1 = psp.tile([128, 1024], f32, tag="pz1")
            psum_zp = [psum_z0, psum_z1]
            psum_sc = psp.tile([128, 256], f32, tag="psc")
            psum_mw = psp.tile([128, 512], f32, tag="pmw")

            def emit_ids(ps, xe, close):
                # identity matmuls fold the x projection into psum (group start)
                for m in range(4):
                    for chv in range(2):
                        co = 1024 * m + 512 * chv
                        nc.tensor.matmul(
                            ps[32 * m:32 * m + NB, 512 * chv:512 * chv + 512],
                            ident[0:NB, 0:NB], xe[0:NB, co:co + 512],
                            start=True, stop=close,
                            tile_position=(0, 32 * m))

            def emit_z_stream(ps, lhsT_of_kk, w_sb_of_kk, with_align,
                              al_t=None):
                # kk-outer, chv-inner: per col group the two chv matmuls share
                # one stationary load (bass skips the redundant LDWEIGHTS)
                for kk in range(8):
                    lh = lhsT_of_kk(kk)
                    for m in range(4):
                        for chv in range(2):
                            co = 1024 * m + 512 * chv
                            nc.tensor.matmul(
                                ps[32 * m:32 * m + NB, 512 * chv:512 * chv + 512],
                                lh, w_sb_of_kk(kk)[:, co:co + 512],
                                start=False,
                                stop=(kk == 7 and not with_align),
                                tile_position=(0, 32 * m))
                if with_align:
                    for m in range(4):
                        for p in range(2):
                            for chv in range(2):
                                co = 1024 * m + 512 * chv
                                nc.tensor.matmul(
                                    ps[32 * m:32 * m + NB, 512 * chv:512 * chv + 512],
                                    alTall[:, p, al_t, :],
                                    MemWca[:, p, co:co + 512],
                                    start=False, stop=(p == 1),
                                    tile_position=(0, 32 * m))

            # x precompute first: its staging pools need the space the big
            # weight pools occupy later.
            x_precompute_all([
                (enc_idx, enc_emb, Wxe, be_t, RE, Xe_d),
                (dec_idx, dec_emb, Wxdx, bd_t, RD, Xd_d),
            ])

            # Whcomb: 6 chunks prefetched during the encoder (gpsimd queue
            # is idle); the last 2 chunks load once Wca's space frees up.
            whcp = scn.enter_context(tc.tile_pool(name="whc", bufs=1))
            whc_a = whcp.tile([128, 6, G4], bf)
            whc_b = None

            def whc_of_kk(kk):
                return whc_a[:, kk, :] if kk < 6 else whc_b[:, kk - 6, :]

            # ---------------- encoder ----------------
            with ExitStack() as ec:
                encp = ec.enter_context(tc.tile_pool(name="enc", bufs=1))
                whe_sb = encp.tile([128, 8, G4], bf)
                for kk in range(8):
                    nc.scalar.dma_start(out=whe_sb[:, kk, :],
                                        in_=Whe[128 * kk:128 * (kk + 1), :])

                nc.vector.memset(C2[:], 0.0)

                nc.sync.dma_start(out=xe_pp[0][:], in_=Xe_d[0:NB, :])
                emit_ids(psum_zp[0], xe_pp[0], close=True)
                for t in range(TS):
                    ps = psum_zp[t % 2]
                    if t + 1 < TS:
                        nc.sync.dma_start(out=xe_pp[(t + 1) % 2][:],
                                          in_=Xe_d[NB * (t + 1):NB * (t + 2), :])
                    if t > 0:
                        emit_z_stream(ps,
                                      (lambda kk, _t=t: memT[:, kk, _t - 1, :]),
                                      (lambda kk: whe_sb[:, kk, :]),
                                      with_align=False)
                    # next step's id matmuls go in front of the tail so they
                    # fill the PE gap (they only need the x tile)
                    if t + 1 < TS:
                        emit_ids(psum_zp[(t + 1) % 2], xe_pp[(t + 1) % 2],
                                 close=False)
                    gate_tail(ps, (lambda h, _t=t:
                                   memT[:, 4 * h:4 * h + 4, _t, :]))
                    # prefetch most of Whcomb on the idle gpsimd queue
                    # (last 2 chunks wait for the Wca space at the transition)
                    if stage != "enc" and t == 40:
                        for kk in range(6):
                            nc.gpsimd.dma_start(
                                out=whc_a[:, kk, :],
                                in_=Whcomb[128 * kk:128 * (kk + 1), :])

                if debug:
                    nc.sync.dma_start(out=dbg["memT"][:], in_=memT[:])
                    nc.sync.dma_start(out=dbg["c_enc"][:], in_=C2[:])

            # ---------------- transition: keys, MemWca, MemWaC ----------------
            m_dec = _re.match(r"dec(\d+)$", stage)
            TD_RUN = int(m_dec.group(1)) if m_dec else TD
            if stage != "enc":
                decp = scn.enter_context(tc.tile_pool(name="dec", bufs=1))

                memQ = decp.tile([128, 8, 2, 128], bf)

                with ExitStack() as c3:
                    wmp = c3.enter_context(tc.tile_pool(name="wmp", bufs=1))
                    wm_sb = wmp.tile([128, 8, U], bf)
                    wm_src = Wm_t[:].rearrange("(k p) c -> p k c", k=8)
                    for ko in range(8):
                        nc.gpsimd.dma_start(
                            out=wm_sb[:, :, 128 * ko:128 * (ko + 1)],
                            in_=wm_src[:, :, 128 * ko:128 * (ko + 1)])
                    # keysT = (memT @ Wm')^T, stored batch-major [p, kk, b, s].
                    # 4 rotating psum accumulators (z banks are idle here) keep
                    # independent chains in flight so LDWEIGHTS stays hidden.
                    rot = [psum_mw[:, 0:256], psum_z0[:, 0:256],
                           psum_z0[:, 512:768], psum_z1[:, 0:256]]
                    for ko in range(8):
                        pa = rot[ko % 4]
                        for kk in range(8):
                            nc.tensor.matmul(pa,
                                             wm_sb[:, kk, 128 * ko:128 * (ko + 1)],
                                             memT[:, kk, :, :],
                                             start=(kk == 0), stop=(kk == 7))
                        nc.vector.tensor_copy(
                            keysT[:, ko],
                            pa.rearrange("p (s b) -> p b s", b=NB))

                    # memQ[:, kk, p, 64q+s] = memT[:, kk, s, 2p+q]
                    for kk in range(8):
                        for p in range(2):
                            nc.vector.tensor_copy(
                                memQ[:, kk, p, :].rearrange("p (q s) -> p q s", q=2),
                                memT[:, kk, :, 2 * p:2 * p + 2].rearrange(
                                    "p s q -> p q s"))

                with ExitStack() as c3b:
                    wcap2 = c3b.enter_context(tc.tile_pool(name="wca2", bufs=1))
                    wca_sb = wcap2.tile([128, 8, G4], bf)
                    # column-block-major: the first MemWca chain (c8=0) only
                    # needs cols 0:512 of every kk chunk, so it starts after
                    # ~1MB instead of the full 8MB load
                    wca_src = Wca_t[:].rearrange("(k p) c -> p k c", k=8)
                    for c8 in range(8):
                        nc.gpsimd.dma_start(
                            out=wca_sb[:, :, 512 * c8:512 * (c8 + 1)],
                            in_=wca_src[:, :, 512 * c8:512 * (c8 + 1)])
                    rot2 = [psum_mw[:], psum_z0[:, 0:512],
                            psum_z0[:, 512:1024], psum_z1[:, 0:512]]
                    for c8 in range(8):
                        for p in range(2):
                            pa = rot2[(2 * c8 + p) % 4]
                            for kk in range(8):
                                nc.tensor.matmul(
                                    pa, memQ[:, kk, p, :],
                                    wca_sb[:, kk, 512 * c8:512 * (c8 + 1)],
                                    start=(kk == 0), stop=(kk == 7))
                            nc.vector.tensor_copy(
                                MemWca[:, p, 512 * c8:512 * (c8 + 1)], pa)

                whcp2 = scn.enter_context(tc.tile_pool(name="whc2", bufs=1))
                whc_b = whcp2.tile([128, 2, G4], bf)
                # decoder x tiles can load as soon as the encoder stops
                # touching the ping-pong buffers
                nc.sync.dma_start(out=xe_pp[0][:], in_=Xd_d[0:NB, :])
                nc.sync.dma_start(out=xe_pp[1][:], in_=Xd_d[NB:2 * NB, :])

                with ExitStack() as c3c:
                    wacp = c3c.enter_context(tc.tile_pool(name="wacp", bufs=1))
                    wac_sb = wacp.tile([128, 8, U], bf)
                    for kk in range(8):
                        nc.gpsimd.dma_start(out=wac_sb[:, kk, :],
                                            in_=WaC_t[128 * kk:128 * (kk + 1), :])
                    rot3 = [psum_mw[:], psum_z0[:, 0:512],
                            psum_z0[:, 512:1024], psum_z1[:, 0:512]]
                    for p in range(2):
                        for c2_ in range(2):
                            pa = rot3[(2 * p + c2_) % 4]
                            for kk in range(8):
                                nc.tensor.matmul(
                                    pa, memQ[:, kk, p, :],
                                    wac_sb[:, kk, 512 * c2_:512 * (c2_ + 1)],
                                    start=(kk == 0), stop=(kk == 7))
                            nc.vector.tensor_copy(
                                MemWaC[:, p, 512 * c2_:512 * (c2_ + 1)], pa)

                if debug:
                    nc.sync.dma_start(out=dbg["keysT"][:], in_=keysT[:])
                    nc.sync.dma_start(out=dbg["MemWca"][:], in_=MemWca[:])

                # ---------------- decoder scan ----------------
                nc.vector.memset(alTall[:], 0.0)

                rsums = decp.tile([128, NB], f32)
                rmask = decp.tile([128, NB], f32)
                rsD = decp.tile([128, 1], f32)
                # rmask[96+p, b] = 1 iff p == b (diag selector)
                nc.vector.tensor_copy(rmask[96:128, :], ident[96:128, 96:96 + NB])

                exp_sc = None
                align_bf = None
                dve_t = None
                attnT = None
                wah_sb = None

                def softmax_emit(t):
                    # scores in psum_sc rows 96:100 -> alTall[:, :, t, :]
                    nc.scalar.activation(exp_sc[96:128, :], psum_sc[96:128, :], AF.Exp)
                    for b in range(NB):
                        nc.vector.reduce_sum(rsums[96:128, b:b + 1],
                                             exp_sc[96:128, 64 * b:64 * (b + 1)],
                                             axis=AX.X)
                    nc.vector.tensor_mul(rsums[96:128, :], rsums[96:128, :],
                                         rmask[96:128, :])
                    nc.vector.reduce_sum(rsD[96:128, :], rsums[96:128, :], axis=AX.X)
                    nc.vector.reciprocal(rsD[96:128, :], rsD[96:128, :])
                    nc.vector.tensor_scalar(align_bf[96:128, :], exp_sc[96:128, :],
                                            rsD[96:128, 0:1], None, op0=ALU.mult)
                    nc.vector.transpose(dve_t[96:128, :], align_bf[96:128, :])
                    # diag value align_b[32h+r] sits at dve_t[96+r, 32*(2b+h)+b]
                    for b in range(NB):
                        p, q = b // 2, b % 2
                        for hh in range(2):
                            cc = 32 * (2 * b + hh) + b
                            nc.vector.tensor_copy(
                                alTall[64 * q + 32 * hh:64 * q + 32 * hh + 32,
                                       p, t, b:b + 1],
                                dve_t[96:128, cc:cc + 1])

                def attn_chunk_mms(j, kos):
                    c0, c1 = CHUNKS[j]
                    cw = (c1 - c0) * NB
                    for ko in kos:
                        pa = psum_mw[:, 0:cw]
                        for kk in range(8):
                            nc.tensor.matmul(
                                pa, wah_sb[:, kk, 128 * ko:128 * (ko + 1)],
                                HdecT[:, kk, 1 + c0:1 + c1, :],
                                start=(kk == 0), stop=False)
                        for p in range(2):
                            nc.tensor.matmul(
                                pa,
                                MemWaC[:, p, 128 * ko:128 * (ko + 1)],
                                alTall[:, p, c0:c1, :].rearrange(
                                    "p t b -> p (t b)"),
                                start=False, stop=(p == 1))
                        nc.vector.tensor_copy(attnT[:, ko, 0:cw], pa)

                def attn_chunk_fin(j):
                    c0, c1 = CHUNKS[j]
                    cw = (c1 - c0) * NB
                    nc.gpsimd.dma_start(
                        out=aginC[j][:].rearrange("k p c -> p k c"),
                        in_=attnT[:, :, 0:cw])
                    nc.gpsimd.collective_compute(
                        "AllGather", ALU.bypass,
                        ins=[aginC[j][:]], outs=[agoutC[j][:]],
                        replica_groups=[list(range(NC))])
                    for r in range(NC):
                        nc.gpsimd.dma_start(
                            out=sb_ag[:, r, :, c0:c1, :],
                            in_=agoutC[j][r].rearrange("k p (t b) -> p k t b",
                                                       b=NB))

                def attn_chunk(j):
                    attn_chunk_mms(j, range(8))
                    attn_chunk_fin(j)

                # chunks 0-2 spread their matmul bursts over 3 steps' tail
                # gaps; the last two chunks stay immediate (tail-critical)
                attn_sched = {}
                for j, (c0, c1) in enumerate(CHUNKS[:3]):
                    attn_sched[c1 - 1] = (j, [0, 1, 2])
                    attn_sched[c1] = (j, [3, 4, 5])
                    attn_sched[c1 + 1] = (j, [6, 7], 'fin')

                def scores_emit(t):
                    for kk in range(8):
                        nc.tensor.matmul(
                            psum_sc[96:96 + NB, :],
                            HdecT[:, kk, t + 1, :],
                            keysT[:, kk].rearrange("p b s -> p (b s)"),
                            start=(kk == 0), stop=(kk == 7),
                            tile_position=(0, 96))

                # streamed t=0 weights (Whd0) in a scoped pool
                with ExitStack() as c4:
                    w0p = c4.enter_context(tc.tile_pool(name="w0", bufs=3))
                    w0_tiles = []
                    for kk in range(8):
                        w0 = w0p.tile([128, G4], bf, tag="w0")
                        nc.gpsimd.dma_start(out=w0[:],
                                            in_=Whd0[128 * kk:128 * (kk + 1), :])
                        w0_tiles.append(w0)
                    # less-urgent weights load after the z_0 stream weights
                    for kk in range(6, 8):
                        nc.gpsimd.dma_start(
                            out=whc_b[:, kk - 6, :],
                            in_=Whcomb[128 * kk:128 * (kk + 1), :])

                    ps = psum_zp[0]
                    emit_ids(ps, xe_pp[0], close=False)
                    emit_z_stream(ps,
                                  (lambda kk: memT[:, kk, TS - 1, :]),
                                  (lambda kk: w0_tiles[kk]),
                                  with_align=False)
                    emit_ids(psum_zp[1], xe_pp[1], close=False)
                    gate_tail(ps, (lambda h: HdecT[:, 4 * h:4 * h + 4, 1, :]))
                    scores_emit(0)

                # softmax scratch + attn staging + WaH + gathered activations
                # (allocated after the w0 pool frees its space)
                dec2p = scn.enter_context(tc.tile_pool(name="dec2", bufs=1))
                exp_sc = dec2p.tile([128, 256], f32)
                align_bf = dec2p.tile([128, 256], bf)
                dve_t = dec2p.tile([128, 256], bf)
                attnT = dec2p.tile([128, 8, 64], bf)   # per-chunk staging
                wah_sb = dec2p.tile([128, 8, U], bf)
                # wah rides the idle gpsimd queue; needed first at t=15
                for kk in range(8):
                    nc.gpsimd.dma_start(out=wah_sb[:, kk, :],
                                        in_=WaH_t[128 * kk:128 * (kk + 1), :])
                softmax_emit(0)

                for t in range(1, TD_RUN):
                    ps = psum_zp[t % 2]
                    if t + 1 < TD_RUN:
                        nc.sync.dma_start(out=xe_pp[(t + 1) % 2][:],
                                          in_=Xd_d[NB * (t + 1):NB * (t + 2), :])
                    emit_z_stream(ps,
                                  (lambda kk, _t=t: HdecT[:, kk, _t, :]),
                                  whc_of_kk,
                                  with_align=True, al_t=t - 1)
                    if t + 1 < TD_RUN:
                        emit_ids(psum_zp[(t + 1) % 2], xe_pp[(t + 1) % 2],
                                 close=False)
                    gate_tail(ps, (lambda h, _t=t:
                                   HdecT[:, 4 * h:4 * h + 4, _t + 1, :]))
                    scores_emit(t)
                    softmax_emit(t)
                    if stage == "full":
                        if t in attn_sched:
                            ent = attn_sched[t]
                            attn_chunk_mms(ent[0], ent[1])
                            if len(ent) > 2:
                                attn_chunk_fin(ent[0])
                        elif (t + 1) in [c1 for _, c1 in CHUNKS[3:]]:
                            attn_chunk(
                                3 + [c1 for _, c1 in CHUNKS[3:]].index(t + 1))

                if debug:
                    nc.sync.dma_start(out=dbg["HallT"][:], in_=HdecT[:])
                    nc.sync.dma_start(out=dbg["alTall"][:], in_=alTall[:])

        # ------- projection (sb_ag filled by the chunked AllGather) -------
        if stage == "full":
            with ExitStack() as c2:
                ppd = c2.enter_context(tc.tile_pool(name="projd", bufs=4))
                ps4 = c2.enter_context(tc.tile_pool(name="projps", bufs=1,
                                                    space="PSUM"))
                wfp = c2.enter_context(tc.tile_pool(name="wfc", bufs=1))
                # all of Wf resident: one stationary load serves all 8 vocab
                # chunks of a row tile (LDWEIGHTS amortized 8x)
                wf_all = wfp.tile([128, 8, VSH], bf)
                wfs_src = Wfs[:].rearrange("(k p) c -> p k c", k=8)
                for sc in range(8):
                    nc.scalar.dma_start(
                        out=wf_all[:, :, 500 * sc:500 * (sc + 1)],
                        in_=wfs_src[:, :, 500 * sc:500 * (sc + 1)])
                bf_all = wfp.tile([128, VSH], f32)
                nc.scalar.dma_start(out=bf_all[:],
                                    in_=bfs[:].to_broadcast([128, VSH]))
                pj_t = [ps4.tile([128, 500], f32, name=f"pj{i}")
                        for i in range(8)]
                for th in range(2):
                    for r in range(NC):
                        t0 = 32 * th
                        t1 = min(t0 + 32, TD)
                        rr = (t1 - t0) * NB
                        r0 = 252 * r + NB * t0
                        lhs = [sb_ag[:, r, kk, t0:t1, :].rearrange(
                                   "p t b -> p (t b)") for kk in range(8)]
                        # sc-outer: each vocab chunk's accumulation group
                        # closes early so its bias-add and output DMA overlap
                        # the next chunk's matmuls
                        for sc in range(8):
                            for kk in range(8):
                                nc.tensor.matmul(
                                    pj_t[sc][:rr, :], lhs[kk],
                                    wf_all[:, kk, 500 * sc:500 * (sc + 1)],
                                    start=(kk == 0), stop=(kk == 7))
                            st = ppd.tile([128, 500], f32, tag="st")
                            nc.vector.tensor_add(
                                st[:rr, :], pj_t[sc][:rr, :],
                                bf_all[:rr, 500 * sc:500 * (sc + 1)])
                            nc.sync.dma_start(
                                out=logits[r0:r0 + rr,
                                           500 * sc:500 * (sc + 1)],
                                in_=st[:rr, :])

        if stage != "full":
            # partial-stage dummy output so the NEFF has its ExternalOutput written
            st0 = gp.tile([1, 4], f32, tag="dummy")
            nc.vector.tensor_copy(st0[:], tga[0:1, 0:4])
            nc.sync.dma_start(out=logits[0:1, 0:4], in_=st0[:])

    nc.finalize()
    return nc, dbg


_CACHE = {}


def _get_nc(stage="full", debug=False):
    key = (stage, debug)
    if key not in _CACHE:
        _CACHE[key] = _build_nc(stage, debug)
    return _CACHE[key]


def run_cores(inputs, stage="full", debug=False, trace=False):
    from concourse.bass_utils import run_bass_kernel_spmd
    shared, per_core = _prep_host(inputs)
    nc, dbg = _get_nc(stage, debug)
    in_maps = []
    for k in range(NC):
        m = dict(shared)
        m.update(per_core[k])
        in_maps.append(m)
    return run_bass_kernel_spmd(nc, in_maps, core_ids=list(range(NC)), trace=trace)


def unshard(outs):
    full = np.concatenate(outs, axis=1)                     # [2016, 32000]
    # rows ordered (r, t, b_local); batch b = 4*r + b_local
    full = full.reshape(NC, TD, NB, VT).transpose(0, 2, 1, 3).reshape(B, TD, VT)
    return np.ascontiguousarray(full.astype(np.float32))


def kernel(**inputs):
    res = run_cores(inputs, stage="full")
    outs = [np.asarray(r["logits"]) for r in res.results]   # [2016, 4000] each
    return unshard(outs)
